# revision 13
# baseline (speedup 1.0000x reference)
"""Sparse-attention head kernel for Trainium2, data-parallel over batch on 8 cores.

Math per batch b (see reference):
  q,k,v = x @ W{q,k,v}.T + b{q,k,v}          # [T, 64]
  qg    = q[keep]                            # [K=T/2, 64]
  att   = softmax(mask(qg @ k.T / sqrt(C)))  # [K, T], row i allows t <= keep[i]
  out   = att @ v                            # [K, 64]

Bias folding: scores (qg+bq)@(k+bk).T differ from (qg+bq)@k.T by a per-row
constant (qg+bq)@bk, which softmax cancels — so k is projected WITHOUT bias,
q WITH bias, v WITH bias.

Device strategy (per core, one batch):
  - host uploads xT [C,T] bf16 plus xgT [C,1024] bf16 = xT columns gathered by
    keep[:1024] (the quadratic part; keep[1024:] is exactly t=3072..4095, so
    those q rows project straight from the xT tail).
  - all projections are W-stationary (transposed outputs), streaming xT:
      ktvt [128,T]: rows 0:64 = kT, 64:128 = vT   (lhsT = [Wk|Wv] per c-chunk)
      qgt2 [128,K]: both halves = qgT + bq        (lhsT = [Wq|Wq] -> free dup)
    kT is duplicated into partitions 64:127 (kt_hi) by an SBUF-SBUF DMA; vT is
    transposed back to natural [t,64] tiles by the DMA XBAR transpose, then
    widened to [t,65] (ones column) for the softmax denominator trick.
  - S_T[t,q] = kT.T @ qgT runs as PE row-tiled PAIRS: two t-blocks concurrently
    on tile_position (0,0)/(64,0) (contraction is only hs=64), 2x throughput
    and mutually hidden LDWEIGHTS.
  - exp on the scalar engine reads the pair's 2 PSUM banks in ONE activation
    (the scalar engine is the pacing engine: ~6.1M exps is the kernel floor).
  - PV out_T[65,q] = vext.T @ E is row-split into t-halves 0:64/64:128 run
    concurrently on tiles (0,0)/(64,0), accumulating into two PSUM halves that
    a DVE add folds at wave end.
  - per-wave epilogue: PE-transpose out_T -> [q,65], divide by denominator, DMA.
All matmul inputs bf16 (fp32 PSUM accumulation); epilogue in fp32.
"""

import math
import os

if "JAX_PLATFORMS" not in os.environ:
    os.environ["JAX_PLATFORMS"] = "axon,cpu"

import numpy as np
import ml_dtypes

B, T, C = 8, 4096, 1024
HS = 64
KQ = T // 2          # 2048 gathered query rows
KQUAD = 1024         # keep rows 0:1024 are the quadratic (gathered) part
NCORES = 8
SCALE = float(C) ** -0.5
QC = 512             # attention wave width (q columns)
SEG = 512            # projection t-segment
BF16 = ml_dtypes.bfloat16
NQC = KQ // QC       # 4
NSEG = T // SEG      # 8


def _keep_indices(t):
    a = math.ceil(t / 4)
    keep = [t - 1 - x for x in range(a)]
    keep += [t - 1 - math.ceil(3 / a * (x - a) ** 2 + a) for x in range(a, math.ceil(t / 2))]
    return np.array(list(reversed(keep)), dtype=np.int64)


KEEP = _keep_indices(T)  # [KQ], ascending; KEEP[1024:] == 3072..4095

# Static block classification at [t=128] x [q=128] granularity.
_NT = T // 128   # 32
_NJ = KQ // 128  # 16
_FULL, _BOUND, _DEAD = 0, 1, 2
_BLOCK_KIND = np.empty((_NT, _NJ), dtype=np.int64)
_MASK_IDX = {}
for _tb in range(_NT):
    for _j in range(_NJ):
        qlo = KEEP[_j * 128]
        qhi = KEEP[_j * 128 + 127]
        if 128 * _tb + 127 <= qlo:
            _BLOCK_KIND[_tb, _j] = _FULL
        elif 128 * _tb > qhi:
            _BLOCK_KIND[_tb, _j] = _DEAD
        else:
            _BLOCK_KIND[_tb, _j] = _BOUND
            _MASK_IDX[(_tb, _j)] = len(_MASK_IDX)
_NMASK = len(_MASK_IDX)

# t-blocks needed per q-chunk; leading-dead j-subblocks per (qc, tb)
_NTB_QC = [int(KEEP[qc * QC + QC - 1]) // 128 + 1 for qc in range(NQC)]


def _alive_j0(qc, tb):
    for jj in range(QC // 128):
        if _BLOCK_KIND[tb, qc * (QC // 128) + jj] != _DEAD:
            return jj
    return QC // 128


def _host_masks():
    m = np.zeros((128, _NMASK * 128), dtype=np.float32)
    for (tb, j), idx in _MASK_IDX.items():
        tvals = 128 * tb + np.arange(128)[:, None]
        kvals = KEEP[j * 128:(j + 1) * 128][None, :]
        m[:, idx * 128:(idx + 1) * 128] = (tvals <= kvals).astype(np.float32)
    return m.astype(BF16)


_prog_cache = {}
TRACE = False          # set by test harness to collect an NTFF profile
TRACE_KW = {}
LAST_RESULTS = None    # BassKernelResults of the most recent kernel() call


def _build_program():
    import concourse.bass as bass
    import concourse.mybir as mybir
    import concourse.tile as tile
    from concourse import bacc
    from concourse.masks import make_identity

    dt = mybir.dt
    f32, bf16 = dt.float32, dt.bfloat16
    Alu = mybir.AluOpType
    Act = mybir.ActivationFunctionType

    nc = bacc.Bacc("TRN2", target_bir_lowering=False, debug=False,
                   enable_partition_id=False)

    xt_d = nc.dram_tensor("xt", [C, T], bf16, kind="ExternalInput").ap()
    xgt_d = nc.dram_tensor("xgt", [C, KQUAD], bf16, kind="ExternalInput").ap()
    wpack_d = nc.dram_tensor("wpack", [128, 8 * 256], f32, kind="ExternalInput").ap()
    bias2_d = nc.dram_tensor("bias2", [128, 2], f32, kind="ExternalInput").ap()
    masks_d = nc.dram_tensor("masks", [128, _NMASK * 128], bf16, kind="ExternalInput").ap()
    out_d = nc.dram_tensor("out", [KQ, HS], f32, kind="ExternalOutput").ap()

    with tile.TileContext(nc) as tc:
        with (
            tc.tile_pool(name="const", bufs=1) as constp,
            tc.tile_pool(name="xt", bufs=1) as xtp,
            tc.tile_pool(name="proj", bufs=1) as projp,
            tc.tile_pool(name="psS", bufs=2, space="PSUM") as psS,
            tc.tile_pool(name="psO", bufs=1, space="PSUM") as psOp,
            tc.tile_pool(name="psP", bufs=2, space="PSUM") as psP,
            tc.tile_pool(name="work", bufs=2) as workp,
            tc.tile_pool(name="ework", bufs=6) as ep,
        ):
            # ---- persistent tensors ----
            xt_big = xtp.tile([128, 8 * T], bf16, name="xt_big")
            xgt_big = xtp.tile([128, 8 * KQUAD], bf16, name="xgt_big")
            ktvt = projp.tile([128, T], bf16, name="ktvt")       # 0:64 kT, 64:128 vT
            kt_hi = projp.tile([128, T], bf16, name="kt_hi")     # 64:128 = kT dup
            qgt2 = projp.tile([128, KQ], bf16, name="qgt2")      # both halves qgT
            vext = projp.tile([128, _NT * 65], bf16, name="vext")

            def xt_sl(c, lo, hi):
                return xt_big[:, c * T + lo: c * T + hi]

            def xgt_sl(c, lo, hi):
                return xgt_big[:, c * KQUAD + lo: c * KQUAD + hi]

            # ---- input DMA triggers ----
            # Trigger-issue is ~0.7us each and a queue drains ~20GB/s, so the
            # early t-segments get fine (512-col) triggers for latency while
            # the tail gets coarse ones; spread across scalar (head only,
            # before exp starts), sync, and gpsimd (SWDGE) so no single queue
            # serializes the issue.
            def xt_trig(eng, lo, hi, cs=range(8)):
                for c in cs:
                    eng.dma_start(out=xt_sl(c, lo, hi),
                                  in_=xt_d[c * 128:(c + 1) * 128, lo:hi])

            def xgt_trig(eng, lo, hi, cs=range(8)):
                for c in cs:
                    eng.dma_start(out=xgt_sl(c, lo, hi),
                                  in_=xgt_d[c * 128:(c + 1) * 128, lo:hi])

            # scalar head (idle until the first exp):
            xt_trig(nc.scalar, 0, 512, range(0, 4))
            xgt_trig(nc.scalar, 0, 512, range(0, 4))
            # sync: rest of seg0/xgt-lo, then seg1..seg3 fine
            xt_trig(nc.sync, 0, 512, range(4, 8))
            xgt_trig(nc.sync, 0, 512, range(4, 8))
            xt_trig(nc.sync, 512, 1024)
            xt_trig(nc.sync, 1024, 1536)
            xt_trig(nc.sync, 1536, 2048)

            # ---- constants (SWDGE so HWDGE queues stay on the bulk loads) ----
            wpack_sb = constp.tile([128, 8 * 256], bf16)
            nc.gpsimd.dma_start(out=wpack_sb, in_=wpack_d)
            bias2_sb = constp.tile([128, 2], f32)
            nc.gpsimd.dma_start(out=bias2_sb, in_=bias2_d)
            mask_big = constp.tile([128, _NMASK * 128], bf16)
            nc.gpsimd.dma_start(out=mask_big, in_=masks_d)
            ident_f = constp.tile([128, 128], f32)
            make_identity(nc, ident_f)
            ident_b = constp.tile([128, 128], bf16)
            make_identity(nc, ident_b)
            # ones columns of vext (positions 64 mod 65), set once
            vext3 = vext.rearrange("p (s c) -> p s c", c=65)
            nc.gpsimd.memset(vext3[:, :, 64:65], 1.0)
            # gpsimd: xgt-hi (qc1 queries, needed ~mid-kernel)
            xgt_trig(nc.gpsimd, 512, 1024)

            def w_kv(c):
                return wpack_sb[:, c * 256: c * 256 + 128]

            def w_qq(c):
                return wpack_sb[:, c * 256 + 128: c * 256 + 256]

            # ---- projection emitters ----
            def emit_kv_seg(si):
                lo = si * SEG
                ps = psP.tile([128, SEG], f32, name="ps_kv", tag="pp")
                for c in range(8):
                    nc.tensor.matmul(ps, lhsT=w_kv(c), rhs=xt_sl(c, lo, lo + SEG),
                                     start=(c == 0), stop=(c == 7))
                # kT (no bias) + vT (+bv) -> bf16 SBUF in one pass
                nc.vector.tensor_scalar(
                    out=ktvt[:, lo:lo + SEG], in0=ps,
                    scalar1=bias2_sb[:, 0:1], scalar2=None, op0=Alu.add)
                # kT dup to partitions 64:127 (SBUF->SBUF, software DGE)
                nc.gpsimd.dma_start(out=kt_hi[64:128, lo:lo + SEG],
                                    in_=ktvt[0:64, lo:lo + SEG])
                # vT -> natural v: PE transposes in the (64,0) row-group, so
                # they overlap row-0 S/PV matmuls
                for tb in range(4 * si, 4 * si + 4):
                    pst = psP.tile([128, HS], bf16, name="ps_vt", tag="pp")
                    nc.tensor.matmul(pst, lhsT=ktvt[64:128, tb * 128:(tb + 1) * 128],
                                     rhs=ident_b[64:128, 64:128],
                                     is_transpose=True, tile_position=(64, 0))
                    nc.vector.tensor_copy(vext3[:, tb:tb + 1, 0:64], pst)

            def emit_qq(qc):
                ps = psP.tile([128, QC], f32, name="ps_qq", tag="pp")
                if qc < 2:
                    src = lambda c: xgt_sl(c, qc * QC, (qc + 1) * QC)
                else:
                    lo = 3072 + (qc - 2) * QC
                    src = lambda c: xt_sl(c, lo, lo + QC)
                for c in range(8):
                    nc.tensor.matmul(ps, lhsT=w_qq(c), rhs=src(c),
                                     start=(c == 0), stop=(c == 7))
                nc.vector.tensor_scalar(
                    out=qgt2[:, qc * QC:(qc + 1) * QC], in0=ps,
                    scalar1=bias2_sb[:, 1:2], scalar2=None, op0=Alu.add)

            # ---- attention waves ----
            pairs_by_qc = []
            for qc in range(NQC):
                ntb = _NTB_QC[qc]
                pairs_by_qc.append([(2 * m, 2 * m + 1) for m in range(ntb // 2)])

            state = {"qc": 0, "pi": 0, "pv": None, "psO": None, "epi": None}

            def emit_S_pair(qc, tba, tbb):
                q0 = qc * QC
                aA = _alive_j0(qc, tba) * 128
                aB = _alive_j0(qc, tbb) * 128
                ps = psS.tile([128, 2 * QC], f32, name="ps_s")
                nc.tensor.matmul(
                    ps[:, aA:QC],
                    lhsT=ktvt[0:64, tba * 128:(tba + 1) * 128],
                    rhs=qgt2[0:64, q0 + aA:q0 + QC],
                    start=True, stop=True, tile_position=(0, 0))
                nc.tensor.matmul(
                    ps[:, QC + aB:2 * QC],
                    lhsT=kt_hi[64:128, tbb * 128:(tbb + 1) * 128],
                    rhs=qgt2[64:128, q0 + aB:q0 + QC],
                    start=True, stop=True, tile_position=(64, 0))
                amin = min(aA, aB)
                e_t = ep.tile([128, 2 * QC], bf16, name="e_t")
                nc.scalar.activation(e_t[:, amin:2 * QC], ps[:, amin:2 * QC],
                                     Act.Exp, scale=SCALE)
                for tb, off, a in ((tba, 0, aA), (tbb, QC, aB)):
                    for jj in range(a // 128, QC // 128):
                        j = qc * (QC // 128) + jj
                        if _BLOCK_KIND[tb, j] == _BOUND:
                            midx = _MASK_IDX[(tb, j)]
                            o = off + jj * 128
                            nc.vector.tensor_tensor(
                                out=e_t[:, o:o + 128], in0=e_t[:, o:o + 128],
                                in1=mask_big[:, midx * 128:(midx + 1) * 128],
                                op=Alu.mult)
                return (qc, tba, tbb, e_t, aA, aB)

            def emit_PV(pv):
                qc, tba, tbb, e_t, aA, aB = pv
                ntb = _NTB_QC[qc]
                psO = state["psO"]
                for tb, off, a in ((tba, 0, aA), (tbb, QC, aB)):
                    first = tb == 0
                    last = tb == ntb - 1
                    nc.tensor.matmul(
                        psO[:, a:QC], lhsT=vext[0:64, tb * 65:(tb + 1) * 65],
                        rhs=e_t[0:64, off + a:off + QC],
                        start=first, stop=last, tile_position=(0, 0))
                    nc.tensor.matmul(
                        psO[:, QC + a:2 * QC], lhsT=vext[64:128, tb * 65:(tb + 1) * 65],
                        rhs=e_t[64:128, off + a:off + QC],
                        start=first, stop=last, tile_position=(64, 0))

            def emit_epilogue_phase1(qc):
                psO = state["psO"]
                oth = workp.tile([65, QC], f32, name="oth", tag="oth")
                nc.vector.tensor_copy(oth, psO[:, QC:2 * QC])
                ot = workp.tile([65, QC], f32, name="ot", tag="ot")
                nc.vector.tensor_tensor(out=ot, in0=psO[:, 0:QC],
                                        in1=oth, op=Alu.add)
                state["psO"] = None
                return (qc, ot)

            def emit_epilogue_phase2(epi):
                qc, ot = epi
                q0 = qc * QC
                out4 = workp.tile([128, (QC // 128) * HS], f32,
                                  name="out4", tag="out4")
                for jj in range(QC // 128):
                    psx = psP.tile([128, 65], f32, name="ps_x", tag="pp")
                    nc.tensor.transpose(psx, ot[:, jj * 128:(jj + 1) * 128],
                                        ident_f[0:65, 0:65])
                    rec = workp.tile([128, 1], f32, name="rec", tag="rec")
                    nc.vector.reciprocal(rec, psx[:, HS:HS + 1])
                    nc.vector.tensor_scalar(
                        out=out4[:, jj * HS:(jj + 1) * HS], in0=psx[:, 0:HS],
                        scalar1=rec[:, :1], scalar2=None, op0=Alu.mult)
                out_view = out_d[q0:q0 + QC, :].rearrange("(j p) d -> p j d", p=128)
                nc.sync.dma_start(out=out_view,
                                  in_=out4.rearrange("p (j d) -> p j d",
                                                     j=QC // 128))

            qq_ready_si = [0, 0, 6, 7]
            qq_emitted = [False] * NQC

            def pump(si):
                # emit all attention work whose inputs exist after t-seg si
                while state["qc"] < NQC:
                    qc = state["qc"]
                    if not qq_emitted[qc]:
                        if qq_ready_si[qc] > si:
                            return
                        emit_qq(qc)
                        qq_emitted[qc] = True
                    pairs = pairs_by_qc[qc]
                    if state["pi"] < len(pairs):
                        tba, tbb = pairs[state["pi"]]
                        if tbb > 4 * si + 3:
                            return  # t-blocks not projected yet
                        if state["psO"] is None:
                            state["psO"] = psOp.tile([65, 2 * QC], f32, name="ps_o")
                        pv = emit_S_pair(qc, tba, tbb)
                        if state["pv"] is not None:
                            emit_PV(state["pv"])
                        state["pv"] = pv
                        state["pi"] += 1
                        if state["epi"] is not None:
                            emit_epilogue_phase2(state["epi"])
                            state["epi"] = None
                    else:
                        if state["pv"] is not None:
                            emit_PV(state["pv"])
                            state["pv"] = None
                        state["epi"] = emit_epilogue_phase1(qc)
                        state["qc"] += 1
                        state["pi"] = 0
                if state["epi"] is not None:
                    emit_epilogue_phase2(state["epi"])
                    state["epi"] = None

            # ---- main schedule ----
            for si in range(NSEG):
                emit_kv_seg(si)
                if si + 4 < NSEG:
                    # late xt segments stream via SWDGE while sync drains the
                    # early fine-grained ones
                    xt_trig(nc.gpsimd, (si + 4) * SEG, (si + 5) * SEG)
                pump(si)
            pump(NSEG)

    nc.compile()
    return nc


def _get_program():
    if "nc" not in _prog_cache:
        _prog_cache["nc"] = _build_program()
    return _prog_cache["nc"]


def _host_pack(Wq, bq, Wk, bk, Wv, bv):
    WqT = np.asarray(Wq, dtype=np.float32).T  # [C, 64]
    WkT = np.asarray(Wk, dtype=np.float32).T
    WvT = np.asarray(Wv, dtype=np.float32).T
    wpack = np.empty((128, 8 * 256), dtype=np.float32)
    for c in range(8):
        sl = slice(c * 128, (c + 1) * 128)
        wpack[:, c * 256:c * 256 + 64] = WkT[sl]
        wpack[:, c * 256 + 64:c * 256 + 128] = WvT[sl]
        wpack[:, c * 256 + 128:c * 256 + 192] = WqT[sl]
        wpack[:, c * 256 + 192:c * 256 + 256] = WqT[sl]
    bias2 = np.zeros((128, 2), dtype=np.float32)
    bias2[64:128, 0] = np.asarray(bv, dtype=np.float32)   # k gets NO bias
    bias2[0:64, 1] = np.asarray(bq, dtype=np.float32)
    bias2[64:128, 1] = np.asarray(bq, dtype=np.float32)
    return wpack, bias2


def kernel(x, Wq, bq, Wk, bk, Wv, bv):
    from concourse.bass_utils import run_bass_kernel_spmd

    x = np.asarray(x, dtype=np.float32)
    wpack, bias2 = _host_pack(Wq, bq, Wk, bk, Wv, bv)
    masks = _host_masks()
    keep_quad = KEEP[:KQUAD]

    nc = _get_program()
    in_maps = []
    for b in range(NCORES):
        xt = np.ascontiguousarray(x[b].T).astype(BF16)       # [C, T]
        xgt = np.ascontiguousarray(xt[:, keep_quad])         # [C, 1024]
        in_maps.append({
            "xt": xt,
            "xgt": xgt,
            "wpack": wpack,
            "bias2": bias2,
            "masks": masks,
        })
    res = run_bass_kernel_spmd(nc, in_maps, core_ids=list(range(NCORES)),
                               trace=TRACE, **TRACE_KW)
    global LAST_RESULTS
    LAST_RESULTS = res
    out = np.stack([res.results[b]["out"] for b in range(NCORES)], axis=0)
    return out.astype(np.float32)
